# revision 6
# baseline (speedup 1.0000x reference)
"""Trainium2 Bass kernel for the ContrastiveModel loss.

Math (per batch b):
    z1 = proj(X1[b]), z2 = proj(X2[b]);  proj(x) = elu(x@W1.T+b1)@W2.T+b2
    z1n, z2n = L2-normalized rows
    E11 = exp(z1n z1n^T / tau), E12 = exp(z1n z2n^T / tau), E22 likewise
    l1 = sum_l [log(rowsum(E11)+rowsum(E12)-diag(E11)) - log(diag(E12))]
    l2 = sum_l [log(rowsum(E22)+colsum(E12)-diag(E22)) - log(diag(E12))]
    loss = mean_b 0.5*(l1+l2)

Sharding: 8 cores, 2 per batch; each core computes a 2048-row block of the
three sim matrices. Everything is computed in transposed [D, L] layout so
the contraction dim lands on SBUF partitions. The host rolls the L axis by
the shard offset so a single SPMD program serves all cores, and finishes
the tiny log/sum reductions in float64.
"""

import numpy as np

import concourse.bass as bass
import concourse.mybir as mybir
import concourse.tile as tile
from concourse import bacc
from concourse.bass_utils import run_bass_kernel_spmd

F32 = mybir.dt.float32
BF16 = mybir.dt.bfloat16
AF = mybir.ActivationFunctionType
ALU = mybir.AluOpType

B, L, D = 4, 4096, 256
NCORES = 8
SHARD = L // 2            # rows of the sim matrices per core
NT = SHARD // 128         # 16 l-tiles per core
NMC = L // 512            # 8 moving chunks of 512
HALF = L // 2             # 2048-wide ACT groups (4 PSUM banks)


def _dma(nc, out, in_):
    nc.sync.dma_start(out=out, in_=in_)


def _projection_phase(nc, pools, xdram, w1s, w2s, b1s, b2s, zt):
    """zt[:, dt, :] (f32 [128, 2, L]) = (W2 @ elu(W1 @ X.T + b1) + b2) d-tile dt."""
    xpool, hspool, ppool_h, ppool_z = pools
    for c in range(NMC):
        cs = slice(c * 512, (c + 1) * 512)
        xt = xpool.tile([128, 2, 512], F32, name="xt", tag="xt")
        for dt in range(2):
            _dma(nc, xt[:, dt, :], xdram[dt, :, cs])
        hp = ppool_h.tile([128, 2, 512], F32, name="hp", tag="hp")
        for pt in range(2):
            for dt in range(2):
                nc.tensor.matmul(
                    hp[:, pt, :],
                    lhsT=w1s[dt][:, pt * 128:(pt + 1) * 128],
                    rhs=xt[:, dt, :],
                    start=(dt == 0), stop=(dt == 1),
                )
        # elu(v) = min(exp(v) - 1, relu(v)), v = hp + b1
        e_sb = hspool.tile([128, 2, 512], F32, name="e_sb", tag="e_sb")
        r_sb = hspool.tile([128, 2, 512], F32, name="r_sb", tag="r_sb")
        h_sb = hspool.tile([128, 2, 512], F32, name="h_sb", tag="h_sb")
        for pt in range(2):
            nc.scalar.activation(e_sb[:, pt, :], hp[:, pt, :], AF.Exp,
                                 bias=b1s[:, pt:pt + 1], scale=1.0)
            nc.vector.tensor_scalar(out=r_sb[:, pt, :], in0=hp[:, pt, :],
                                    scalar1=b1s[:, pt:pt + 1], scalar2=0.0,
                                    op0=ALU.add, op1=ALU.max)
            nc.vector.scalar_tensor_tensor(out=h_sb[:, pt, :], in0=e_sb[:, pt, :],
                                           scalar=-1.0, in1=r_sb[:, pt, :],
                                           op0=ALU.add, op1=ALU.min)
        zp = ppool_z.tile([128, 2, 512], F32, name="zp", tag="zp")
        for dt in range(2):
            for k in range(2):
                nc.tensor.matmul(
                    zp[:, dt, :],
                    lhsT=w2s[k][:, dt * 128:(dt + 1) * 128],
                    rhs=h_sb[:, k, :],
                    start=(k == 0), stop=(k == 1),
                )
            nc.vector.tensor_scalar(out=zt[:, dt, cs], in0=zp[:, dt, :],
                                    scalar1=b2s[:, dt:dt + 1], scalar2=None,
                                    op0=ALU.add)


def _build_bass():
    nc = bacc.Bacc("TRN2", target_bir_lowering=False, debug=False,
                   num_devices=NCORES)
    x1t = nc.dram_tensor("x1t", [2, 128, L], F32, kind="ExternalInput").ap()
    x2t = nc.dram_tensor("x2t", [2, 128, L], F32, kind="ExternalInput").ap()
    w1t = nc.dram_tensor("w1t", [2, 128, D], F32, kind="ExternalInput").ap()
    w2t = nc.dram_tensor("w2t", [2, 128, D], F32, kind="ExternalInput").ap()
    b1v = nc.dram_tensor("b1v", [2, 128], F32, kind="ExternalInput").ap()
    b2v = nc.dram_tensor("b2v", [2, 128], F32, kind="ExternalInput").ap()

    r11o = nc.dram_tensor("r11", [128, NT], F32, kind="ExternalOutput").ap()
    r12o = nc.dram_tensor("r12", [128, NT], F32, kind="ExternalOutput").ap()
    r22o = nc.dram_tensor("r22", [128, NT], F32, kind="ExternalOutput").ap()
    cs12o = nc.dram_tensor("cs12", [1, L], F32, kind="ExternalOutput").ap()
    u12o = nc.dram_tensor("u12", [1, L], F32, kind="ExternalOutput").ap()
    ns1o = nc.dram_tensor("ns1", [1, L], F32, kind="ExternalOutput").ap()
    ns2o = nc.dram_tensor("ns2", [1, L], F32, kind="ExternalOutput").ap()

    with tile.TileContext(nc) as tc:
        with (
            tc.tile_pool(name="consts", bufs=1) as consts,
            tc.tile_pool(name="zbig", bufs=1) as zbig,
        ):
            # constants
            w1s = [consts.tile([128, D], F32, name=f"w1_{dt}") for dt in range(2)]
            w2s = [consts.tile([128, D], F32, name=f"w2_{dt}") for dt in range(2)]
            for dt in range(2):
                _dma(nc, w1s[dt][:, :], w1t[dt])
                _dma(nc, w2s[dt][:, :], w2t[dt])
            b1s = consts.tile([128, 2], F32, name="b1s")
            b2s = consts.tile([128, 2], F32, name="b2s")
            for pt in range(2):
                _dma(nc, b1s[:, pt:pt + 1], b1v[pt].rearrange("(p o) -> p o", o=1))
                _dma(nc, b2s[:, pt:pt + 1], b2v[pt].rearrange("(p o) -> p o", o=1))
            ones = consts.tile([128, 1], F32, name="ones")
            nc.vector.memset(ones, 1.0)

            # normalized bf16 z's, persistent through the sim phase
            zb1 = zbig.tile([128, 2, L], BF16, name="zb1")
            zb2 = zbig.tile([128, 2, L], BF16, name="zb2")

            with tc.tile_pool(name="zfull", bufs=1) as zfull:
                zt1 = zfull.tile([128, 2, L], F32, name="zt1")
                zt2 = zfull.tile([128, 2, L], F32, name="zt2")

                # ------------- projection -------------
                with (
                    tc.tile_pool(name="xpool", bufs=2) as xpool,
                    tc.tile_pool(name="hspool", bufs=2) as hspool,
                    tc.tile_pool(name="ppool_h", bufs=2, space="PSUM") as ppool_h,
                    tc.tile_pool(name="ppool_z", bufs=2, space="PSUM") as ppool_z,
                ):
                    pools = (xpool, hspool, ppool_h, ppool_z)
                    _projection_phase(nc, pools, x1t, w1s, w2s, b1s, b2s, zt1)
                    _projection_phase(nc, pools, x2t, w1s, w2s, b1s, b2s, zt2)

                # ------------- norms, diag dots, normalize -------------
                with (
                    tc.tile_pool(name="nvec", bufs=1) as nvec,
                    tc.tile_pool(name="sqpool", bufs=3) as sqpool,
                    tc.tile_pool(name="cpsum", bufs=2, space="PSUM") as cpsum,
                ):
                    rn1 = nvec.tile([1, L], F32, name="rn1")
                    rn2 = nvec.tile([1, L], F32, name="rn2")
                    # norms^2 -> dram + rnorm chunks; u12 dots -> dram
                    jobs = ((zt1, zt1, ns1o, rn1), (zt2, zt2, ns2o, rn2),
                            (zt1, zt2, u12o, None))
                    for za, zc, dram_out, rn in jobs:
                        for c in range(NMC):
                            cs = slice(c * 512, (c + 1) * 512)
                            sq = sqpool.tile([128, 2, 512], F32, name="sq", tag="sq")
                            for dt in range(2):
                                nc.vector.tensor_mul(sq[:, dt, :], za[:, dt, cs],
                                                     zc[:, dt, cs])
                            ps = cpsum.tile([1, 512], F32, name="cps", tag="cps")
                            for dt in range(2):
                                nc.tensor.matmul(ps[:, :], lhsT=ones[:, :],
                                                 rhs=sq[:, dt, :],
                                                 start=(dt == 0), stop=(dt == 1))
                            st = sqpool.tile([1, 512], F32, name="st", tag="st")
                            nc.vector.tensor_copy(st[:, :], ps[:, :])
                            _dma(nc, dram_out[0, cs].rearrange("(o l) -> o l", o=1),
                                 st[:, :])
                            if rn is not None:
                                nc.scalar.activation(rn[:, cs], ps[:, :], AF.Sqrt)
                    for rn in (rn1, rn2):
                        nc.vector.reciprocal(rn[:, :], rn[:, :])
                    for zt, rn, zb in ((zt1, rn1, zb1), (zt2, rn2, zb2)):
                        rn_rep = nvec.tile([128, L], F32, name="rn_rep",
                                           tag="rn_rep")
                        nc.gpsimd.partition_broadcast(rn_rep[:, :], rn[:, :])
                        for dt in range(2):
                            nc.vector.tensor_tensor(
                                out=zb[:, dt, :], in0=zt[:, dt, :],
                                in1=rn_rep[:, :], op=ALU.mult)

            # ---------------- similarity phase ----------------
            with (
                tc.tile_pool(name="simpsum", bufs=2, space="PSUM") as simpsum,
                tc.tile_pool(name="e12pool", bufs=3) as e12pool,
                tc.tile_pool(name="scrpool", bufs=2) as scrpool,
                tc.tile_pool(name="accpool", bufs=1) as accpool,
                tc.tile_pool(name="outpool", bufs=1) as outpool,
            ):
                colacc = accpool.tile([128, L], F32, name="colacc")
                nc.vector.memset(colacc, 0.0)
                rparts = {m: accpool.tile([128, NT * 2], F32, name=f"rp{m}")
                          for m in (11, 12, 22)}

                for t in range(NT):
                    ts_ = slice(t * 128, (t + 1) * 128)
                    for mat, lhs_src, rhs_src in ((11, zb1, zb1), (12, zb1, zb2),
                                                  (22, zb2, zb2)):
                        for h in range(2):
                            ps = simpsum.tile([128, HALF], F32, name="ps", tag="ps")
                            for dt in range(2):
                                lhs = lhs_src[:, dt, ts_]
                                for mc in range(4):
                                    mcs = slice(h * HALF + mc * 512,
                                                h * HALF + (mc + 1) * 512)
                                    nc.tensor.matmul(
                                        ps[:, mc * 512:(mc + 1) * 512],
                                        lhsT=lhs, rhs=rhs_src[:, dt, mcs],
                                        start=(dt == 0), stop=(dt == 1),
                                    )
                            acc_slice = rparts[mat][:, t * 2 + h:t * 2 + h + 1]
                            if mat == 12:
                                eb = e12pool.tile([128, HALF], BF16, name="eb",
                                                  tag="eb")
                                nc.scalar.activation(eb[:, :], ps[:, :], AF.Exp,
                                                     scale=2.0, accum_out=acc_slice)
                                nc.vector.tensor_tensor(
                                    out=colacc[:, h * HALF:(h + 1) * HALF],
                                    in0=colacc[:, h * HALF:(h + 1) * HALF],
                                    in1=eb[:, :], op=ALU.add)
                            else:
                                scr = scrpool.tile([128, HALF], BF16, name="scr",
                                                   tag="scr")
                                nc.scalar.activation(scr[:, :], ps[:, :], AF.Exp,
                                                     scale=2.0, accum_out=acc_slice)

                # final reductions + stores
                for mat, out_ap in ((11, r11o), (12, r12o), (22, r22o)):
                    rfin = outpool.tile([128, NT], F32, name=f"rf{mat}")
                    nc.vector.tensor_reduce(
                        out=rfin[:, :],
                        in_=rparts[mat].rearrange("p (t h) -> p t h", h=2),
                        axis=mybir.AxisListType.X, op=ALU.add)
                    _dma(nc, out_ap[:], rfin[:, :])
                colacc3 = colacc.rearrange("p (o l) -> p o l", o=1)
                for c in range(NMC):
                    cs = slice(c * 512, (c + 1) * 512)
                    psbig = simpsum.tile([128, HALF], F32, name="ps", tag="ps")
                    ps = psbig[0:1, 0:512]
                    nc.tensor.matmul(ps, lhsT=ones[:, :],
                                     rhs=colacc3[:, 0, cs], start=True, stop=True)
                    st = outpool.tile([1, 512], F32, name="cst", tag=f"cst{c}")
                    nc.vector.tensor_copy(st[:, :], ps)
                    _dma(nc, cs12o[0, cs].rearrange("(o l) -> o l", o=1), st[:, :])

    nc.compile()
    return nc


_NC_CACHE = None


def _get_nc():
    global _NC_CACHE
    if _NC_CACHE is None:
        _NC_CACHE = _build_bass()
    return _NC_CACHE


def _make_in_maps(X1, X2, W1, b1, W2, b2):
    w1t = np.ascontiguousarray(W1.T).reshape(2, 128, D).astype(np.float32)
    w2t = np.ascontiguousarray(W2.T).reshape(2, 128, D).astype(np.float32)
    b1v = b1.reshape(2, 128).astype(np.float32)
    b2v = b2.reshape(2, 128).astype(np.float32)
    in_maps = []
    for c in range(NCORES):
        b, s = divmod(c, 2)
        x1 = np.roll(np.ascontiguousarray(X1[b].T), -s * SHARD, axis=1)
        x2 = np.roll(np.ascontiguousarray(X2[b].T), -s * SHARD, axis=1)
        in_maps.append({
            "x1t": np.ascontiguousarray(x1).reshape(2, 128, L).astype(np.float32),
            "x2t": np.ascontiguousarray(x2).reshape(2, 128, L).astype(np.float32),
            "w1t": w1t, "w2t": w2t, "b1v": b1v, "b2v": b2v,
        })
    return in_maps


def _finish_host(results):
    """Combine per-core partials into the final scalar loss (float64)."""
    total = 0.0
    for b in range(B):
        c0, c1 = 2 * b, 2 * b + 1
        r11 = np.concatenate([
            results[c0]["r11"].T.reshape(-1), results[c1]["r11"].T.reshape(-1)
        ]).astype(np.float64)
        r12 = np.concatenate([
            results[c0]["r12"].T.reshape(-1), results[c1]["r12"].T.reshape(-1)
        ]).astype(np.float64)
        r22 = np.concatenate([
            results[c0]["r22"].T.reshape(-1), results[c1]["r22"].T.reshape(-1)
        ]).astype(np.float64)
        cs12 = (results[c0]["cs12"].reshape(-1).astype(np.float64) +
                np.roll(results[c1]["cs12"].reshape(-1).astype(np.float64), SHARD))
        ns1 = results[c0]["ns1"].reshape(-1).astype(np.float64)
        ns2 = results[c0]["ns2"].reshape(-1).astype(np.float64)
        u12 = results[c0]["u12"].reshape(-1).astype(np.float64)

        n1 = np.maximum(np.sqrt(ns1), 1e-12)
        n2 = np.maximum(np.sqrt(ns2), 1e-12)
        d11 = ns1 / (n1 * n1)          # ~1.0, matches reference diag
        d22 = ns2 / (n2 * n2)
        s12d = u12 / (n1 * n2)
        denom1 = r11 + r12 - np.exp(2.0 * d11)
        denom2 = r22 + cs12 - np.exp(2.0 * d22)
        l1 = np.sum(np.log(denom1)) - 2.0 * np.sum(s12d)
        l2 = np.sum(np.log(denom2)) - 2.0 * np.sum(s12d)
        total += 0.5 * (l1 + l2)
    return np.float32(total / B)


def kernel(X1, X2, W1, b1, W2, b2):
    X1 = np.asarray(X1, dtype=np.float32)
    X2 = np.asarray(X2, dtype=np.float32)
    W1 = np.asarray(W1, dtype=np.float32)
    b1 = np.asarray(b1, dtype=np.float32)
    W2 = np.asarray(W2, dtype=np.float32)
    b2 = np.asarray(b2, dtype=np.float32)
    nc = _get_nc()
    in_maps = _make_in_maps(X1, X2, W1, b1, W2, b2)
    res = run_bass_kernel_spmd(nc, in_maps, core_ids=list(range(NCORES)))
    return _finish_host(res.results)
